# revision 4
# baseline (speedup 1.0000x reference)
"""TRN2 Bass kernel for nn_MultiHeadAttention_79714593014244.

Reference math (per token n, NOT sequence attention):
    Q = x @ W_q, K = x @ W_k, V = x @ W_v          (x: [N, 4096])
    S[n] = Q[n] @ K[n].T        over heads          ([32, 32] per token)
    A[n] = softmax(S[n], axis=-1)
    y[n] = A[n] @ V[n]
    out = y.reshape(N, 4096) @ W_o

Sharding: pure data-parallel over tokens across 8 cores (attention is
per-token, so no cross-core communication). Each core handles 1024
tokens with all four weights streamed from HBM once per projection.

Per-core plan (all matmuls in float32r: ~bf16 speed, ~2e-4 accuracy):
  Phase 0: DMA x tile-wise, PE-transpose to xT resident in SBUF
           (layout [128 d, 32 c-chunks, 1024 tok]).
  Phase A: for W in (W_q, W_k, W_v): stream W column-chunks, compute
           QT/KT/VT = W.T @ x.T in [feat, tok] layout, spill to DRAM.
           Feature chunks of 128 coincide with heads (head h = rows
           h*128..h*128+128), so QT chunk h is [d, tok] for head h.
  Phase B: per 512-token half, per 128-token macro: load QT/KT/VT
           slices; for each group of 4 tokens:
             - 4 col-packed matmuls  S[4tx32h, 32g] (contract d=128)
             - batched softmax over g (reduce_max -> Exp+bias+accum ->
               reciprocal -> scale)
             - 4 PE transposes A -> AT[4tx32g, 32h]
             - 4 PE transposes V -> Vt[4tx32g, 128d]
             - 4 row-packed matmuls yT[128d, 4tx32h] (contract g=32)
             - strided copy into yt_half [128 d, 32 h, 512 tok]
  Phase C: per half: stream W_o column-chunks, out = y @ W_o.
"""

import os

import numpy as np

import concourse.bass as bass
import concourse.tile as tile
from concourse import bacc, mybir
from concourse.bass_utils import run_bass_kernel_spmd

N_CORES = 8
N_TOKENS = 8192
DIM = 4096
H = 32  # heads
D = 128  # head dim
KC = DIM // 128  # contraction chunks (32)
TOK = N_TOKENS // N_CORES  # tokens per core (1024)
HALF = 512  # tokens per B+C fusion block
MACRO = 128  # tokens per attention slice load
F32R = mybir.dt.float32r
F32 = mybir.dt.float32

_NC_CACHE = {}


def _build_nc():
    nc = bacc.Bacc(None, target_bir_lowering=False)

    x_d = nc.dram_tensor("x", [TOK, DIM], F32R, kind="ExternalInput")
    wq_d = nc.dram_tensor("wq", [DIM, DIM], F32R, kind="ExternalInput")
    wk_d = nc.dram_tensor("wk", [DIM, DIM], F32R, kind="ExternalInput")
    wv_d = nc.dram_tensor("wv", [DIM, DIM], F32R, kind="ExternalInput")
    wo_d = nc.dram_tensor("wo", [DIM, DIM], F32R, kind="ExternalInput")
    id_d = nc.dram_tensor("ident", [128, 128], F32R, kind="ExternalInput")
    out_d = nc.dram_tensor("out", [TOK, DIM], F32, kind="ExternalOutput")

    qt_d = nc.dram_tensor("qt_i", [H, D, TOK], F32R, kind="Internal")
    kt_d = nc.dram_tensor("kt_i", [H, D, TOK], F32R, kind="Internal")
    vt_d = nc.dram_tensor("vt_i", [H, D, TOK], F32R, kind="Internal")

    with tile.TileContext(nc) as tc:
        with tc.tile_pool(name="consts", bufs=1) as constp:
            id_sb = constp.tile([128, 128], F32R)
            nc.sync.dma_start(out=id_sb[:, :], in_=id_d[:, :])

            # ---------- Phase 0 + A: projections ----------
            with tc.tile_pool(name="xT", bufs=1) as xtp:
                xT = xtp.tile([128, KC, TOK], F32R)  # 128 KB/partition

                with (
                    tc.tile_pool(name="xload", bufs=2) as xlp,
                    tc.tile_pool(name="tps", bufs=4, space="PSUM") as tps,
                ):
                    for tt in range(TOK // 128):
                        xl = xlp.tile([128, DIM], F32R)
                        nc.sync.dma_start(
                            out=xl[:, :], in_=x_d[tt * 128 : (tt + 1) * 128, :]
                        )
                        for c in range(KC):
                            ps = tps.tile([128, 128], F32R)
                            nc.tensor.transpose(
                                ps[:, :], xl[:, c * 128 : (c + 1) * 128], id_sb[:, :]
                            )
                            nc.vector.tensor_copy(
                                xT[:, c, tt * 128 : (tt + 1) * 128], ps[:, :]
                            )

                with (
                    tc.tile_pool(name="wb", bufs=3) as wbp,
                    tc.tile_pool(name="stA", bufs=3) as stp,
                    tc.tile_pool(name="aps", bufs=3, space="PSUM") as aps,
                ):
                    for w_d, o_d in ((wq_d, qt_d), (wk_d, kt_d), (wv_d, vt_d)):
                        w_r = w_d[:, :].rearrange("(kc c) f -> c kc f", c=128)
                        for F in range(KC):
                            wb = wbp.tile([128, KC, 128], F32R, tag="wb")
                            nc.sync.dma_start(
                                out=wb[:, :, :],
                                in_=w_r[:, :, F * 128 : (F + 1) * 128],
                            )
                            for th in range(TOK // 512):
                                ps = aps.tile([128, 512], F32, tag="aps")
                                for kc in range(KC):
                                    nc.tensor.matmul(
                                        ps[:, :],
                                        wb[:, kc, :],
                                        xT[:, kc, th * 512 : (th + 1) * 512],
                                        start=(kc == 0),
                                        stop=(kc == KC - 1),
                                    )
                                st = stp.tile([128, 512], F32R, tag="st")
                                nc.scalar.copy(st[:, :], ps[:, :])
                                nc.sync.dma_start(
                                    out=o_d[F, :, th * 512 : (th + 1) * 512],
                                    in_=st[:, :],
                                )

            # ---------- Phase B + C per 512-token half ----------
            qt_r = qt_d[:, :, :].rearrange("h d t -> d h t")
            kt_r = kt_d[:, :, :].rearrange("h d t -> d h t")
            vt_r = vt_d[:, :, :].rearrange("h d t -> d h t")
            wo_r = wo_d[:, :].rearrange("(kc c) f -> c kc f", c=128)

            with tc.tile_pool(name="yt", bufs=1) as ytp:
                for half in range(TOK // HALF):
                    yt = ytp.tile([128, KC, HALF], F32R, tag="yt")
                    h0 = half * HALF

                    with (
                        tc.tile_pool(name="qkv", bufs=2) as qkvp,
                        tc.tile_pool(name="smax", bufs=4) as smp,
                        tc.tile_pool(name="att", bufs=3) as attp,
                        tc.tile_pool(name="bps", bufs=2, space="PSUM") as bps,
                        tc.tile_pool(name="bps2", bufs=2, space="PSUM") as bps2,
                    ):
                        for mt in range(HALF // MACRO):
                            m0 = h0 + mt * MACRO
                            q_sl = qkvp.tile([128, H, MACRO], F32R, tag="q")
                            k_sl = qkvp.tile([128, H, MACRO], F32R, tag="k")
                            v_sl = qkvp.tile([128, H, MACRO], F32R, tag="v")
                            nc.sync.dma_start(
                                out=q_sl[:, :, :], in_=qt_r[:, :, m0 : m0 + MACRO]
                            )
                            nc.sync.dma_start(
                                out=k_sl[:, :, :], in_=kt_r[:, :, m0 : m0 + MACRO]
                            )
                            nc.sync.dma_start(
                                out=v_sl[:, :, :], in_=vt_r[:, :, m0 : m0 + MACRO]
                            )
                            for g4 in range(MACRO // 4):
                                # 4 tokens per group, packed along the FREE
                                # axis (fp32r matmul/transpose outputs must
                                # sit at psum partition base 0).
                                n0 = g4 * 4
                                s_ps = bps.tile([32, 128], F32, tag="s")
                                vt_ps = bps.tile([32, 512], F32R, tag="vt")
                                at_ps = bps2.tile([32, 128], F32R, tag="at")
                                y_ps = bps2.tile([128, 128], F32, tag="y")
                                for t in range(4):
                                    nc.tensor.matmul(
                                        s_ps[:, 32 * t : 32 * t + 32],
                                        q_sl[:, :, n0 + t],
                                        k_sl[:, :, n0 + t],
                                        start=True,
                                        stop=True,
                                        skip_group_check=True,
                                    )
                                    nc.tensor.transpose(
                                        vt_ps[:, 128 * t : 128 * t + 128],
                                        v_sl[:, :, n0 + t],
                                        id_sb[:, :],
                                    )
                                # softmax over g: [32 h, 4 t x 32 g]
                                negmax = smp.tile([32, 4], F32, tag="nm")
                                denom = smp.tile([32, 4], F32, tag="dn")
                                recip = smp.tile([32, 4], F32, tag="rc")
                                e_sb = attp.tile([32, 128], F32, tag="e")
                                a_sb = attp.tile([32, 128], F32R, tag="a")
                                vt_sb = attp.tile([32, 512], F32R, tag="vts")
                                at_sb = attp.tile([32, 128], F32R, tag="ats")
                                nc.vector.reduce_max(
                                    negmax[:, :],
                                    s_ps[:, :].rearrange("p (t g) -> p t g", t=4),
                                    axis=mybir.AxisListType.X,
                                    negate=True,
                                )
                                for t in range(4):
                                    nc.scalar.activation(
                                        e_sb[:, 32 * t : 32 * t + 32],
                                        s_ps[:, 32 * t : 32 * t + 32],
                                        mybir.ActivationFunctionType.Exp,
                                        bias=negmax[:, t : t + 1],
                                        accum_out=denom[:, t : t + 1],
                                    )
                                nc.vector.reciprocal(recip[:, :], denom[:, :])
                                for t in range(4):
                                    nc.vector.tensor_scalar_mul(
                                        a_sb[:, 32 * t : 32 * t + 32],
                                        e_sb[:, 32 * t : 32 * t + 32],
                                        recip[:, t : t + 1],
                                    )
                                nc.vector.tensor_copy(vt_sb[:, :], vt_ps[:, :])
                                for t in range(4):
                                    nc.tensor.transpose(
                                        at_ps[:, 32 * t : 32 * t + 32],
                                        a_sb[:, 32 * t : 32 * t + 32],
                                        id_sb[0:32, 0:32],
                                    )
                                nc.vector.tensor_copy(at_sb[:, :], at_ps[:, :])
                                for t in range(4):
                                    nc.tensor.matmul(
                                        y_ps[:, 32 * t : 32 * t + 32],
                                        vt_sb[:, 128 * t : 128 * t + 128],
                                        at_sb[:, 32 * t : 32 * t + 32],
                                        start=True,
                                        stop=True,
                                        skip_group_check=True,
                                    )
                                # yT_ps free = (t, h) -> yt free = (h, tok)
                                src = y_ps[:, :].rearrange(
                                    "p (t h) -> p h t", t=4
                                )
                                dst_off = mt * MACRO + n0
                                dst = yt[:, :, dst_off : dst_off + 4]
                                nc.vector.tensor_copy(dst, src)

                    with (
                        tc.tile_pool(name="wob", bufs=2) as wop,
                        tc.tile_pool(name="stC", bufs=3) as stc,
                        tc.tile_pool(name="cps", bufs=3, space="PSUM") as cps,
                    ):
                        for fo in range(DIM // 256):
                            wob = wop.tile([128, KC, 256], F32R, tag="wob")
                            nc.sync.dma_start(
                                out=wob[:, :, :],
                                in_=wo_r[:, :, fo * 256 : (fo + 1) * 256],
                            )
                            for tt in range(HALF // 128):
                                ps = cps.tile([128, 256], F32, tag="cps")
                                for kc in range(KC):
                                    nc.tensor.matmul(
                                        ps[:, :],
                                        yt[:, kc, tt * 128 : (tt + 1) * 128],
                                        wob[:, kc, :],
                                        start=(kc == 0),
                                        stop=(kc == KC - 1),
                                    )
                                st = stc.tile([128, 256], F32, tag="stc")
                                nc.scalar.copy(st[:, :], ps[:, :])
                                nc.sync.dma_start(
                                    out=out_d[
                                        h0 + tt * 128 : h0 + (tt + 1) * 128,
                                        fo * 256 : (fo + 1) * 256,
                                    ],
                                    in_=st[:, :],
                                )

    nc.compile()
    return nc


def _get_nc():
    if "nc" not in _NC_CACHE:
        _NC_CACHE["nc"] = _build_nc()
    return _NC_CACHE["nc"]


def kernel(x, W_q, W_k, W_v, W_o):
    x = np.ascontiguousarray(x, dtype=np.float32)
    W_q = np.ascontiguousarray(W_q, dtype=np.float32)
    W_k = np.ascontiguousarray(W_k, dtype=np.float32)
    W_v = np.ascontiguousarray(W_v, dtype=np.float32)
    W_o = np.ascontiguousarray(W_o, dtype=np.float32)

    ident = np.eye(128, dtype=np.float32)

    nc = _get_nc()
    in_maps = []
    for c in range(N_CORES):
        in_maps.append(
            {
                "x": x[c * TOK : (c + 1) * TOK],
                "wq": W_q,
                "wk": W_k,
                "wv": W_v,
                "wo": W_o,
                "ident": ident,
            }
        )
    trace = bool(int(os.environ.get("KERNEL_TRACE", "0")))
    res = run_bass_kernel_spmd(
        nc, in_maps, core_ids=list(range(N_CORES)), trace=trace
    )
    if trace:
        kernel.last_exec_time_ns = res.exec_time_ns
        kernel.last_results = res
    out = np.concatenate([r["out"] for r in res.results], axis=0)
    return np.ascontiguousarray(out, dtype=np.float32)


# revision 5
# speedup vs baseline: 1.1005x; 1.1005x over previous
"""TRN2 Bass kernel for nn_MultiHeadAttention_79714593014244.

Reference math (per token n, NOT sequence attention):
    Q = x @ W_q, K = x @ W_k, V = x @ W_v          (x: [N, 4096])
    S[n] = Q[n] @ K[n].T        over heads          ([32, 32] per token)
    A[n] = softmax(S[n], axis=-1)
    y[n] = A[n] @ V[n]
    out = y.reshape(N, 4096) @ W_o

Sharding: pure data-parallel over tokens across 8 cores (attention is
per-token, so no cross-core communication). Each core handles 1024
tokens with all four weights streamed from HBM once per projection.

Per-core plan (all matmuls in float32r: ~bf16 speed, ~2e-4 accuracy):
  Phase 0: DMA x tile-wise, PE-transpose to xT resident in SBUF
           (layout [128 d, 32 c-chunks, 1024 tok]).
  Phase A: for W in (W_q, W_k, W_v): stream W column-chunks, compute
           QT/KT/VT = W.T @ x.T in [feat, tok] layout, spill to DRAM.
           Feature chunks of 128 coincide with heads (head h = rows
           h*128..h*128+128), so QT chunk h is [d, tok] for head h.
  Phase B: per 512-token half, per 128-token macro: load QT/KT/VT
           slices; for each group of 4 tokens:
             - 4 col-packed matmuls  S[4tx32h, 32g] (contract d=128)
             - batched softmax over g (reduce_max -> Exp+bias+accum ->
               reciprocal -> scale)
             - 4 PE transposes A -> AT[4tx32g, 32h]
             - 4 PE transposes V -> Vt[4tx32g, 128d]
             - 4 row-packed matmuls yT[128d, 4tx32h] (contract g=32)
             - strided copy into yt_half [128 d, 32 h, 512 tok]
  Phase C: per half: stream W_o column-chunks, out = y @ W_o.
"""

import os

import ml_dtypes
import numpy as np

import concourse.bass as bass
import concourse.tile as tile
from concourse import bacc, mybir
from concourse.bass_utils import run_bass_kernel_spmd

N_CORES = 8
N_TOKENS = 8192
DIM = 4096
H = 32  # heads
D = 128  # head dim
KC = DIM // 128  # contraction chunks (32)
TOK = N_TOKENS // N_CORES  # tokens per core (1024)
HALF = 512  # tokens per B+C fusion block
MACRO = 64  # tokens per attention slice load
F32R = mybir.dt.float32r
F32 = mybir.dt.float32
BF16 = mybir.dt.bfloat16

_NC_CACHE = {}


def _build_nc():
    nc = bacc.Bacc(None, target_bir_lowering=False)

    x_d = nc.dram_tensor("x", [TOK, DIM], F32R, kind="ExternalInput")
    wq_d = nc.dram_tensor("wq", [DIM, DIM], F32R, kind="ExternalInput")
    wk_d = nc.dram_tensor("wk", [DIM, DIM], F32R, kind="ExternalInput")
    wv_d = nc.dram_tensor("wv", [DIM, DIM], F32R, kind="ExternalInput")
    wo_d = nc.dram_tensor("wo_bf16", [DIM, DIM], BF16, kind="ExternalInput")
    id_d = nc.dram_tensor("ident", [128, 128], F32R, kind="ExternalInput")
    out_d = nc.dram_tensor("out", [TOK, DIM], F32, kind="ExternalOutput")

    qt_d = nc.dram_tensor("qt_i", [H, D, TOK], F32R, kind="Internal")
    kt_d = nc.dram_tensor("kt_i", [H, D, TOK], F32R, kind="Internal")
    vt_d = nc.dram_tensor("vt_i", [H, D, TOK], F32R, kind="Internal")

    with tile.TileContext(nc) as tc:
        with tc.tile_pool(name="consts", bufs=1) as constp:
            id_sb = constp.tile([128, 128], F32R)
            nc.sync.dma_start(out=id_sb[:, :], in_=id_d[:, :])

            # ---------- Phase 0 + A: projections ----------
            with tc.tile_pool(name="xT", bufs=1) as xtp:
                xT = xtp.tile([128, KC, TOK], F32R)  # 128 KB/partition

                with (
                    tc.tile_pool(name="xload", bufs=2) as xlp,
                    tc.tile_pool(name="tps", bufs=4, space="PSUM") as tps,
                ):
                    for tt in range(TOK // 128):
                        xl = xlp.tile([128, DIM], F32R)
                        nc.sync.dma_start(
                            out=xl[:, :], in_=x_d[tt * 128 : (tt + 1) * 128, :]
                        )
                        for c in range(KC):
                            ps = tps.tile([128, 128], F32R)
                            nc.tensor.transpose(
                                ps[:, :], xl[:, c * 128 : (c + 1) * 128], id_sb[:, :]
                            )
                            nc.vector.tensor_copy(
                                xT[:, c, tt * 128 : (tt + 1) * 128], ps[:, :]
                            )

                with (
                    tc.tile_pool(name="wb", bufs=3) as wbp,
                    tc.tile_pool(name="stA", bufs=3) as stp,
                    tc.tile_pool(name="aps", bufs=3, space="PSUM") as aps,
                ):
                    for w_d, o_d in ((wq_d, qt_d), (wk_d, kt_d), (wv_d, vt_d)):
                        w_r = w_d[:, :].rearrange("(kc c) f -> c kc f", c=128)
                        for F in range(KC):
                            wb = wbp.tile([128, KC, 128], F32R, tag="wb")
                            nc.sync.dma_start(
                                out=wb[:, :, :],
                                in_=w_r[:, :, F * 128 : (F + 1) * 128],
                            )
                            for th in range(TOK // 512):
                                ps = aps.tile([128, 512], F32, tag="aps")
                                for kc in range(KC):
                                    nc.tensor.matmul(
                                        ps[:, :],
                                        wb[:, kc, :],
                                        xT[:, kc, th * 512 : (th + 1) * 512],
                                        start=(kc == 0),
                                        stop=(kc == KC - 1),
                                    )
                                st = stp.tile([128, 512], F32R, tag="st")
                                nc.scalar.copy(st[:, :], ps[:, :])
                                nc.sync.dma_start(
                                    out=o_d[F, :, th * 512 : (th + 1) * 512],
                                    in_=st[:, :],
                                )

            # ---------- Phase B + C per 512-token half ----------
            qt_r = qt_d[:, :, :].rearrange("h d t -> d h t")
            kt_r = kt_d[:, :, :].rearrange("h d t -> d h t")
            vt_r = vt_d[:, :, :].rearrange("h d t -> d h t")
            wo_r = wo_d[:, :].rearrange("(kc c) f -> c kc f", c=128)

            with tc.tile_pool(name="yt", bufs=1) as ytp:
                for half in range(TOK // HALF):
                    yt = ytp.tile([128, KC, HALF], BF16, tag="yt")
                    h0 = half * HALF

                    with (
                        tc.tile_pool(name="qkv", bufs=2) as qkvp,
                        tc.tile_pool(name="smax", bufs=4) as smp,
                        tc.tile_pool(name="att", bufs=3) as attp,
                        tc.tile_pool(name="bps", bufs=2, space="PSUM") as bps,
                        tc.tile_pool(name="bps2", bufs=2, space="PSUM") as bps2,
                    ):
                        for mt in range(HALF // MACRO):
                            m0 = h0 + mt * MACRO
                            q_sl = qkvp.tile([128, H, MACRO], F32R, tag="q")
                            k_sl = qkvp.tile([128, H, MACRO], F32R, tag="k")
                            v_sl = qkvp.tile([32, D, MACRO], F32R, tag="v")
                            nc.sync.dma_start(
                                out=q_sl[:, :, :], in_=qt_r[:, :, m0 : m0 + MACRO]
                            )
                            nc.sync.dma_start(
                                out=k_sl[:, :, :], in_=kt_r[:, :, m0 : m0 + MACRO]
                            )
                            nc.sync.dma_start(
                                out=v_sl[:, :, :], in_=vt_d[:, :, m0 : m0 + MACRO]
                            )
                            for g4 in range(MACRO // 4):
                                # 4 tokens per group, packed along the FREE
                                # axis (fp32r matmul/transpose outputs must
                                # sit at psum partition base 0).
                                n0 = g4 * 4
                                s_ps = bps.tile([32, 128], F32, tag="s")
                                at_ps = bps2.tile([32, 128], F32R, tag="at")
                                y_ps = bps2.tile([128, 128], F32, tag="y")
                                for t in range(4):
                                    nc.tensor.matmul(
                                        s_ps[:, 32 * t : 32 * t + 32],
                                        q_sl[:, :, n0 + t],
                                        k_sl[:, :, n0 + t],
                                        start=True,
                                        stop=True,
                                        skip_group_check=True,
                                    )
                                # softmax over g: [32 h, 4 t x 32 g]
                                negmax = smp.tile([32, 4], F32, tag="nm")
                                denom = smp.tile([32, 4], F32, tag="dn")
                                recip = smp.tile([32, 4], F32, tag="rc")
                                e_sb = attp.tile([32, 128], F32, tag="e")
                                a_sb = attp.tile([32, 128], F32R, tag="a")
                                at_sb = attp.tile([32, 128], F32R, tag="ats")
                                nc.vector.reduce_max(
                                    negmax[:, :],
                                    s_ps[:, :].rearrange("p (t g) -> p t g", t=4),
                                    axis=mybir.AxisListType.X,
                                    negate=True,
                                )
                                for t in range(4):
                                    nc.scalar.activation(
                                        e_sb[:, 32 * t : 32 * t + 32],
                                        s_ps[:, 32 * t : 32 * t + 32],
                                        mybir.ActivationFunctionType.Exp,
                                        bias=negmax[:, t : t + 1],
                                        accum_out=denom[:, t : t + 1],
                                    )
                                nc.vector.reciprocal(recip[:, :], denom[:, :])
                                for t in range(4):
                                    nc.vector.tensor_scalar_mul(
                                        a_sb[:, 32 * t : 32 * t + 32],
                                        e_sb[:, 32 * t : 32 * t + 32],
                                        recip[:, t : t + 1],
                                    )
                                for t in range(4):
                                    nc.tensor.transpose(
                                        at_ps[:, 32 * t : 32 * t + 32],
                                        a_sb[:, 32 * t : 32 * t + 32],
                                        id_sb[0:32, 0:32],
                                    )
                                nc.vector.tensor_copy(at_sb[:, :], at_ps[:, :])
                                for t in range(4):
                                    nc.tensor.matmul(
                                        y_ps[:, 32 * t : 32 * t + 32],
                                        v_sl[:, :, n0 + t],
                                        at_sb[:, 32 * t : 32 * t + 32],
                                        start=True,
                                        stop=True,
                                        skip_group_check=True,
                                    )
                                # yT_ps free = (t, h) -> yt free = (h, tok)
                                src = y_ps[:, :].rearrange(
                                    "p (t h) -> p h t", t=4
                                )
                                dst_off = mt * MACRO + n0
                                dst = yt[:, :, dst_off : dst_off + 4]
                                nc.vector.tensor_copy(dst, src)

                    with (
                        tc.tile_pool(name="wob", bufs=2) as wop,
                        tc.tile_pool(name="stC", bufs=3) as stc,
                        tc.tile_pool(name="cps", bufs=3, space="PSUM") as cps,
                    ):
                        for fo in range(DIM // 512):
                            wob = wop.tile([128, KC, 512], BF16, tag="wob")
                            nc.sync.dma_start(
                                out=wob[:, :, :],
                                in_=wo_r[:, :, fo * 512 : (fo + 1) * 512],
                            )
                            for tt in range(HALF // 128):
                                ps = cps.tile([128, 512], F32, tag="cps")
                                for kc in range(KC):
                                    nc.tensor.matmul(
                                        ps[:, :],
                                        yt[:, kc, tt * 128 : (tt + 1) * 128],
                                        wob[:, kc, :],
                                        start=(kc == 0),
                                        stop=(kc == KC - 1),
                                    )
                                st = stc.tile([128, 512], F32, tag="stc")
                                nc.scalar.copy(st[:, :], ps[:, :])
                                nc.sync.dma_start(
                                    out=out_d[
                                        h0 + tt * 128 : h0 + (tt + 1) * 128,
                                        fo * 512 : (fo + 1) * 512,
                                    ],
                                    in_=st[:, :],
                                )

    nc.compile()
    return nc


def _get_nc():
    if "nc" not in _NC_CACHE:
        _NC_CACHE["nc"] = _build_nc()
    return _NC_CACHE["nc"]


def kernel(x, W_q, W_k, W_v, W_o):
    x = np.ascontiguousarray(x, dtype=np.float32)
    W_q = np.ascontiguousarray(W_q, dtype=np.float32)
    W_k = np.ascontiguousarray(W_k, dtype=np.float32)
    W_v = np.ascontiguousarray(W_v, dtype=np.float32)
    W_o = np.ascontiguousarray(W_o, dtype=np.float32)

    ident = np.eye(128, dtype=np.float32)

    wo_bf16 = W_o.astype(ml_dtypes.bfloat16)

    nc = _get_nc()
    in_maps = []
    for c in range(N_CORES):
        in_maps.append(
            {
                "x": x[c * TOK : (c + 1) * TOK],
                "wq": W_q,
                "wk": W_k,
                "wv": W_v,
                "wo_bf16": wo_bf16,
                "ident": ident,
            }
        )
    trace = bool(int(os.environ.get("KERNEL_TRACE", "0")))
    res = run_bass_kernel_spmd(
        nc, in_maps, core_ids=list(range(N_CORES)), trace=trace
    )
    if trace:
        kernel.last_exec_time_ns = res.exec_time_ns
        kernel.last_results = res
    out = np.concatenate([r["out"] for r in res.results], axis=0)
    return np.ascontiguousarray(out, dtype=np.float32)
